# revision 47
# baseline (speedup 1.0000x reference)
"""DCN kernel for 8 trn2 NeuronCores (Bass/Tile), v5.

Math: in eval mode the deep stack (BN -> Linear x3 -> BN each) has no
nonlinearity, so it collapses to a single weight-side vector: the
prediction-head row ph folded back through W3/W2/W1 and the BN affines
gives u0 = a0 * (W1^T a1 W2^T a2 W3^T (a3*ph)) plus a scalar constant K
(the b/c bias terms dotted through the chain). The DCN cross layers are
rank-1, so the whole model reduces per sample to five dot products
D = [x0|numb]^T [cw0, cw1, cw2, px, u0] plus a scalar recurrence.

u0, K, and the S-sums of the G row are pure functions of the WEIGHT
inputs (standard BN-folding / consecutive-linear collapse), so kernel()
folds them on the host. The device keeps every data-dependent step:
  - embedding gathers: bf16 row-padded image (256B stride, 128B
    payload descriptors -- half the f32 bytes), 8 plain gather windows
    of 3 tables (int16 idx limit) plus one merged transpose-mode gather
    for tables 24/25 that lands feature-major tiles directly (the tail
    chunk skips the transpose+copy chain); GPSIMD desc-gen paces the
    stream at ~1.5us/window.
  - PE transposes of x0 into feature-major chunks (bf16, 1 cyc/row),
    PSUM->SBUF copies alternating DVE/ACT.
  - D via 60 stationary-swapped matmuls (stat = x0^T chunk [128,128],
    moving = U [128,5]) accumulating batch-major [128,5] in PSUM.
  - 7-dependent-op cross recurrence on DVE (the cross +1 rides D col 0
    via a constant row; d4+g5+K is hoisted), sigmoid on ACT.

Per core: 512 batch rows, data-parallel over batch; no cross-core
communication (a collective_compute costs a 15us constant in the cost
model, and any weight sharding it would enable saves less than that).
"""

import sys

if "/opt/trn_rl_repo" not in sys.path:
    sys.path.insert(0, "/opt/trn_rl_repo")

import numpy as np
import ml_dtypes

import concourse.bacc as bacc
import concourse.mybir as mybir
import concourse.tile as tile
from concourse.bass_utils import run_bass_kernel_spmd

F32 = mybir.dt.float32
BF16 = mybir.dt.bfloat16
I16 = mybir.dt.int16
AF = mybir.ActivationFunctionType
OP = mybir.AluOpType
AX = mybir.AxisListType

B, F, V, D = 4096, 26, 10000, 64
NCORES = 8
BC = B // NCORES          # 512 rows per core
D0 = F * D                # 1664
KCH = 13                  # gathered 128-wide feature chunks
NCH = 14                  # uniform chunks incl. the numb chunk
DCH = 15                  # D chunks: tables 24/25 split (transpose-gathers)
CCH = 4                   # 128-wide batch chunks per core
EPS = 1e-5
NW = 8                    # plain gather windows (3 tables each)
WT = [3] * 8              # tables per window
ROWSW = [3 * V] * 8
IDXW = [512 * t for t in WT] + [1024]   # + the merged transpose-gather
SLOTW = [n // 16 for n in IDXW]         # gix free-dim slots per window
SLOT_OFF = np.concatenate([[0], np.cumsum(SLOTW)]).tolist()
GIXW = SLOT_OFF[-1]                     # 832
# free-dim position of (window w, table t) blocks in x0s: 12 per full window
POS_W = [12 * w for w in range(8)]
TPOS = [POS_W[t // 3] + (t % 3) * 4 for t in range(24)]

_CACHED = None
_LAST_RES = None


def _dma_gather_raw(nc, out_ap, in_ap, idxs_ap, num_idxs, elem_size,
                    elem_step):
    """DRAM-source non-transpose dma_gather minus the 256B-elem assert.

    Mirrors BassGpSimd.dma_gather: descriptors are elem_size elements
    (128B for bf16 rows) read at elem_step stride (256B, so the
    descriptor stride field stays byte-256 aligned).
    """
    g = nc.gpsimd
    stride_bytes = elem_step * mybir.dt.size(in_ap.dtype)
    assert stride_bytes % 256 == 0
    inst = g.add_instruction(
        mybir.InstDMAGatherAnt(
            name=g.bass.get_next_instruction_name(),
            ins=[
                *g.lower_ap_dma(in_ap, for_custom_bir_dma=True),
                g.lower_ap(idxs_ap),
                g.lower_val_access(g.to_reg(num_idxs)),
            ],
            outs=[g.lower_ap(out_ap)],
            transpose=False,
            num_idxs=num_idxs,
            elem_size=elem_size,
            stride_bytes_256=stride_bytes // 256,
            gen_mode=0,
            single_packet=False,
            queue_num=0,
            sbuf_tokens_per_rank=0,
            sbuf_free_dim_per_rank=0,
            sbuf_free_dim_pad_per_rank=0,
            sbuf_byte_offset=0,
        )
    )
    return inst


def _build():
    nc = bacc.Bacc("TRN2", target_bir_lowering=False,
                   dynamic_dma_scratch_size=65536)

    emb = nc.dram_tensor("emb", [F * V, 2 * D], BF16, kind="ExternalInput")
    gidx = nc.dram_tensor("gidx", [128, GIXW], I16, kind="ExternalInput")
    numb_p = nc.dram_tensor("numb_p", [128, CCH * 14], BF16,
                            kind="ExternalInput")
    u_in = nc.dram_tensor("u_in", [128, DCH * 5], BF16, kind="ExternalInput")
    g_in = nc.dram_tensor("g_in", [128, 8], F32, kind="ExternalInput")
    idb = nc.dram_tensor("idb", [128, 128], BF16, kind="ExternalInput")
    outp = nc.dram_tensor("outp", [128, 64], F32, kind="ExternalOutput")

    with tile.TileContext(nc) as tc:
        with (
            tc.tile_pool(name="big", bufs=1) as big,
            tc.tile_pool(name="sm", bufs=1) as smp,
            tc.tile_pool(name="ts", bufs=1) as tsp,
            tc.tile_pool(name="scr", bufs=4) as scr,
            tc.tile_pool(name="ps_tp", bufs=2, space="PSUM") as ps_tp,
            tc.tile_pool(name="ps_d", bufs=1, space="PSUM") as ps_d,
        ):
            # ---------------- persistent SBUF tiles ----------------
            x0s = big.tile([128, 96, D], BF16)       # [p, pos, d]
            gix = smp.tile([128, GIXW], I16)
            idnb = smp.tile([128, 128], BF16)
            nb = smp.tile([128, CCH, 14], BF16)
            umr = smp.tile([128, DCH, 5], BF16)      # U col layout
            gbs = smp.tile([128, 8], F32)
            ds = smp.tile([128, CCH, 5], F32)
            osb = smp.tile([128, CCH], F32)
            Tkr = [tsp.tile([128, CCH * 128], BF16, tag=f"tk{k}",
                            name=f"tk{k}") for k in range(12)]
            T45 = tsp.tile([128, 2 * CCH * 128], BF16)
            nT = tsp.tile([128, CCH * 128], BF16)

            # ---------------- input DMAs ----------------
            # SP queue: gather idxs first (they gate the Pool desc-gen
            # pipeline, the critical path), then the small folded tensors.
            nc.sync.dma_start(gix[:, 0:SLOT_OFF[1]], gidx[:, 0:SLOT_OFF[1]])
            nc.sync.dma_start(gix[:, SLOT_OFF[1]:], gidx[:, SLOT_OFF[1]:])
            nc.sync.dma_start(umr[:, :, :], u_in[:, :].rearrange(
                "p (k j) -> p k j", k=DCH))
            nc.sync.dma_start(gbs[:, :], g_in[:, :])
            # ACT queue: identity + numb features (small, off-critical).
            nc.scalar.dma_start(idnb[:, :], idb[:, :])
            nc.scalar.dma_start(nb[:, :, :], numb_p[:, :].rearrange(
                "p (c j) -> p c j", c=CCH))

            # load the Sigmoid act-function set before any ACT Copy runs,
            # so the final sigmoid needs no table switch
            nc.scalar.activation(osb[0:1, 0:1], gbs[0:1, 0:1], AF.Sigmoid)

            # ------------- gathers (9 windows of <=3 tables) -------------
            # window w covers tables 3w..3w+WT-1; idx order i=(t*4+c)*128+p
            # so x0s gets [p, pos=12w+4t+c, d]
            for w in range(NW):
                _dma_gather_raw(
                    nc, x0s[:, POS_W[w]:POS_W[w] + 4 * WT[w], :],
                    emb[3 * w * V:3 * w * V + ROWSW[w], 0:D],
                    gix[:, SLOT_OFF[w]:SLOT_OFF[w + 1]],
                    IDXW[w], D, 2 * D)
            # tables 24/25: one transpose-mode gather lands feature-major
            # tiles directly (payload in image halves [pay|0] / [0|pay] ->
            # rows 0:64 / 64:128 hold the data; the other half is zeros)
            nc.gpsimd.dma_gather(
                T45[:, :].rearrange("p (r i) -> p r i", r=1),
                emb[24 * V:26 * V, :], gix[:, SLOT_OFF[8]:SLOT_OFF[9]],
                1024, 1024, 2 * D, transpose=True, single_packet=False)

            # ---- numb transposes into nT (chunk 13; rows 14:128 zero,
            # row 13 = constant 1.0 so D col 0 absorbs the cross +1) ----
            nc.vector.memset(nT[:, :], 0.0)
            ntp = ps_tp.tile([128, 512], BF16, tag="ntp")
            for c in range(CCH):
                nc.tensor.transpose(ntp[0:14, c * 128:(c + 1) * 128],
                                    nb[:, c, :], idnb[:, :])
            nc.vector.tensor_copy(nT[0:14, :], ntp[0:14, :])

            # ---- x0 transposes (bf16, 1 cyc/row) ----
            for k in range(12):
                tp = ps_tp.tile([128, 512], BF16, tag="tp")
                for c in range(CCH):
                    for h in range(2):
                        nc.tensor.transpose(
                            tp[64 * h:64 * h + 64, c * 128:(c + 1) * 128],
                            x0s[:, TPOS[2 * k + h] + c, :], idnb[:, :])
                if k % 2 == 0 or k >= 11:
                    nc.vector.tensor_copy(Tkr[k][:, :], tp[:, :])
                else:
                    nc.scalar.copy(Tkr[k][:, :], tp[:, :])

            # ---- D: stationary-swapped matmuls, batch-major PSUM out ----
            # chunks 12 (table 24) and 14 (table 25) land last: order every
            # accumulation group so only they trail the final gathers
            pd = ps_d.tile([128, CCH * 5], F32, tag="d")

            def dstat(k, c):
                if k < 12:
                    return Tkr[k][:, c * 128:(c + 1) * 128]
                if k == 13:
                    return nT[:, c * 128:(c + 1) * 128]
                off = 0 if k == 12 else 512
                return T45[:, off + c * 128:off + (c + 1) * 128]

            KORD = list(range(12)) + [13, 12, 14]
            for c in range(CCH):
                for i, k in enumerate(KORD):
                    nc.tensor.matmul(pd[:, c * 5:(c + 1) * 5],
                                     dstat(k, c), umr[:, k, :],
                                     start=(i == 0), stop=(i == DCH - 1))
            nc.vector.tensor_copy(
                ds[:, :, :], pd[:, :].rearrange("p (c j) -> p c j", c=CCH))

            # ---------------- cross recurrence + sigmoid ----------------
            # D col 0 already includes the +1 (nT row 13); w = d4 + (g5+K)
            # is hoisted off the serial chain, leaving 7 dependent ops.
            dcol = lambda j: ds[:, :, j:j + 1].rearrange("p c j -> p (c j)")
            gcol = lambda j: gbs[:, j:j + 1]
            s0p1 = dcol(0)
            w = scr.tile([128, CCH], F32, tag="rc")
            nc.vector.tensor_scalar(w[:, :], dcol(4), gcol(5), None, OP.add)
            A1 = scr.tile([128, CCH], F32, tag="rc")
            nc.vector.tensor_mul(A1[:, :], dcol(1), s0p1)
            nc.vector.tensor_scalar(A1[:, :], A1[:, :], gcol(0), 1.0, OP.add,
                                    OP.add)
            A2 = scr.tile([128, CCH], F32, tag="rc")
            nc.vector.tensor_mul(A2[:, :], dcol(2), s0p1)
            nc.vector.tensor_scalar(A2[:, :], A2[:, :], gcol(1), None, OP.add)
            P = scr.tile([128, CCH], F32, tag="rc")
            nc.vector.tensor_mul(P[:, :], dcol(3), s0p1)
            nc.vector.tensor_scalar(P[:, :], P[:, :], gcol(2), None, OP.add)
            nc.vector.tensor_mul(A2[:, :], A2[:, :], A1[:, :])
            nc.vector.tensor_scalar(A2[:, :], A2[:, :], gcol(3), 1.0, OP.add,
                                    OP.add)
            nc.vector.tensor_mul(P[:, :], P[:, :], A1[:, :])
            nc.vector.tensor_scalar(P[:, :], P[:, :], gcol(4), None, OP.add)
            nc.vector.tensor_mul(P[:, :], P[:, :], A2[:, :])
            nc.vector.tensor_add(P[:, :], P[:, :], w[:, :])
            nc.scalar.activation(osb[:, :], P[:, :], AF.Sigmoid)
            nc.sync.dma_start(outp[:, 0:CCH], osb[:, :])

    nc.compile()
    return nc


def _prep_core(cat_c, numb_c):
    """Per-core host layout prep: int16 gather idxs + numb permute."""
    gidx = np.zeros((128, GIXW), np.int16)
    for w in range(NW):
        v = cat_c[:, 3 * w:3 * w + WT[w]].astype(np.int32)  # [512, WT]
        v = v + (np.arange(WT[w], dtype=np.int32) * V)[None, :]
        # slot i = (t*4 + c)*128 + p
        flat = v.reshape(CCH, 128, WT[w]).transpose(2, 0, 1).reshape(-1)
        wrap = flat.reshape(-1, 16).T.astype(np.int16)      # [16, SLOTW]
        gidx[:, SLOT_OFF[w]:SLOT_OFF[w + 1]] = np.tile(wrap, (8, 1))
    # merged transpose-gather: slots 0:512 = table 24, 512:1024 = table 25
    flat = np.concatenate([cat_c[:, 24], cat_c[:, 25] + V]).astype(np.int16)
    wrap = flat.reshape(-1, 16).T
    gidx[:, SLOT_OFF[8]:SLOT_OFF[9]] = np.tile(wrap, (8, 1))
    nbx = np.ones((CCH, 128, 14), np.float32)
    nbx[:, :, 0:13] = numb_c.reshape(CCH, 128, 13)
    nbp = np.ascontiguousarray(nbx.transpose(1, 0, 2)).reshape(
        128, CCH * 14).astype(ml_dtypes.bfloat16)
    return gidx, nbp


def _fold(inputs):
    """Host weight folding (float64): BN affines + deep-stack collapse.

    Returns U [1792, 5] (cw0,cw1,cw2,px,u0 in 14 chunk-major cols) and
    the G row [8] = [cb0*S1, cb0*S2, cb0*Sp, cb1*S2, cb1*Sp, cb2*Sp, K, 0].
    """
    f = lambda k: np.asarray(inputs[k], np.float64)
    aff = lambda p: ((p[0] / np.sqrt(p[3] + EPS)),
                     (p[1] - p[2] * p[0] / np.sqrt(p[3] + EPS)))
    a0, c0 = aff(f("bn0"))
    a1, c1 = aff(f("bn1"))
    a2, c2 = aff(f("bn2"))
    a3, c3 = aff(f("bn3"))
    pw = f("pred_w")[0]
    ph, px = pw[1664:1920], pw[0:1664]

    q3 = a3 * ph
    r2 = f("w3").T @ q3
    q2 = a2 * r2
    r1 = f("w2").T @ q2
    q1 = a1 * r1
    r0 = f("w1").T @ q1                    # [1677]
    u0 = a0 * r0
    K = (f("pred_b")[0] + ph @ c3 + q3 @ f("b3") + q2 @ f("b2")
         + q1 @ f("b1") + r2 @ c2 + r1 @ c1 + r0 @ c0)

    cw = f("cross_w")
    S1, S2, Sp = cw[1].sum(), cw[2].sum(), px.sum()
    cb = f("cross_b")
    # col 5 carries cb2*Sp + K fused (one tensor_scalar in the recurrence)
    grow = np.array([cb[0] * S1, cb[0] * S2, cb[0] * Sp, cb[1] * S2,
                     cb[1] * Sp, cb[2] * Sp + K, 0.0, 0.0], np.float64)

    U = np.zeros((NCH * 128, 5), np.float64)
    U[0:1664, 0] = cw[0]
    U[0:1664, 1] = cw[1]
    U[0:1664, 2] = cw[2]
    U[0:1664, 3] = px
    U[0:1677, 4] = u0
    return U, grow


def kernel(**inputs):
    global _CACHED, _LAST_RES
    if _CACHED is None:
        _CACHED = _build()
    nc = _CACHED

    f32 = lambda k: np.ascontiguousarray(np.asarray(inputs[k], np.float32))
    cat = np.asarray(inputs["cat_features"])

    # embedding image: row-padded bf16 [F*V, 128] (64 payload + 64 pad);
    # table 25 stores [pad | payload] so its transpose-gather lands the
    # data on partitions 64:128 (zeros elsewhere)
    embi = np.zeros((F * V, 2 * D), ml_dtypes.bfloat16)
    embi[:, 0:D] = f32("emb_tables").reshape(F * V, D).astype(
        ml_dtypes.bfloat16)
    embi[25 * V:26 * V, D:2 * D] = embi[25 * V:26 * V, 0:D]
    embi[25 * V:26 * V, 0:D] = 0

    U, grow = _fold(inputs)
    # u_in: [p, k, j] = U15[k][p, j]; chunks 12/14 hold tables 24/25's
    # 64-feature halves (rows 64:128 zero / rows 0:64 zero resp.)
    U15 = np.zeros((DCH, 128, 5), np.float64)
    for k in range(12):
        U15[k] = U[k * 128:(k + 1) * 128]
    U15[12, 0:64] = U[1536:1600]
    U15[13] = U[1664:1792]
    U15[13, 13, 0] = 1.0          # nT row 13 is 1.0: D col 0 gets the +1
    U15[14, 64:128] = U[1600:1664]
    u_img = np.ascontiguousarray(
        U15.transpose(1, 0, 2).reshape(128, DCH * 5)
    ).astype(ml_dtypes.bfloat16)
    g_img = np.broadcast_to(grow.astype(np.float32), (128, 8))

    shared = {
        "emb": embi,
        "u_in": u_img,
        "g_in": np.ascontiguousarray(g_img),
        "idb": np.eye(128, dtype=np.float32).astype(ml_dtypes.bfloat16),
    }
    numb = f32("numb_features")
    in_maps = []
    for i in range(NCORES):
        gidx, nbp = _prep_core(cat[i * BC:(i + 1) * BC],
                               numb[i * BC:(i + 1) * BC])
        in_maps.append({**shared, "gidx": gidx, "numb_p": nbp})

    res = run_bass_kernel_spmd(nc, in_maps, list(range(NCORES)))
    _LAST_RES = res
    out = np.empty((B, 1), np.float32)
    for i in range(NCORES):
        out[i * BC:(i + 1) * BC, 0] = \
            res.results[i]["outp"][:, 0:CCH].T.reshape(BC)
    return out
